# Initial kernel scaffold
#
"""Trainium2 Bass kernel for 3-layer GRU + dual mask heads.

Model (eval): x [32, 512, 513] -> 3x GRUCell(H=512) scan over T -> two linear
heads (513 out) + relu -> normalized masks -> (mask1*x, mask2*x).

Strategy: data-parallel over batch (4 per core, 8 cores). Per core, the
recurrence runs as a layer-wavefront (layer l processes time t=s-l at step s) so
every matmul at a step depends only on states from step s-1. States are kept
transposed ([H, 4] bf16) and used as matmul stationaries; weights stream as the
moving operand. Gate elementwise math runs on stacked [row-block, 512] tiles
(L1 rows 0-3, L2 32-35, L3 64-67 via PE column tiling). gi1 = W_ih1 @ x_t for
all t is precomputed with big matmuls into transposed layout and injected into
the per-step PSUM accumulation through identity matmuls. h' is re-transposed
each step with PE transposes. Heads run as big matmuls at the end.
"""
import sys
import numpy as np

sys.path.insert(0, "/opt/trn_rl_repo")

import ml_dtypes  # noqa: E402
from contextlib import ExitStack  # noqa: E402

import concourse.bass as bass  # noqa: E402
import concourse.tile as tile  # noqa: E402
import bass_rust  # noqa: E402
from concourse import mybir  # noqa: E402
from concourse.bass_utils import run_bass_kernel_spmd  # noqa: E402

B, T_FULL, F_IN, H = 32, 512, 513, 512
G3 = 3 * H  # 1536
N_CORES = 8
B_LOC = B // N_CORES  # 4
F32 = mybir.dt.float32
BF16 = mybir.dt.bfloat16
AF = mybir.ActivationFunctionType

_SPLIT_CNT = [0]


def _split_multi_waits(nc):
    """This walrus build supports only ONE sem-wait per instruction; split
    extra on_wait entries into preceding single-wait NoOps on the same engine."""
    total = 0
    for fn in nc.m.functions:
        for bb in fn.blocks:
            out = []
            changed = False
            for inst in bb.instructions:
                si = getattr(inst, "sync_info", None)
                ow = si.on_wait if si is not None else None
                if ow and len(ow) > 1:
                    extra = list(ow[:-1])
                    del ow[:-1]
                    for w in extra:
                        _SPLIT_CNT[0] += 1
                        total += 1
                        nd = mybir.InstNoOp(
                            name=f"I-wsplit-{_SPLIT_CNT[0]}", ins=[], outs=[],
                            engine=inst.engine,
                        )
                        nd.sync_info = bass_rust.SyncInfo(on_wait=[w], on_update=[])
                        out.append(nd)
                    changed = True
                out.append(inst)
            if changed:
                bb.instructions[:] = out
    return total


def _build(T):
    assert T % 32 == 0
    NTB = B_LOC * T  # rows (t-major: row = 4t+b)
    nc = bass.Bass("TRN2", target_bir_lowering=False, debug=False,
                   num_devices=N_CORES)

    dt_in = {}

    def din(name, shape, dt):
        dt_in[name] = (shape, dt)
        return nc.dram_tensor(name, list(shape), dt, kind="ExternalInput").ap()

    x = din("x", (B_LOC, T, F_IN), F32)
    wih1T = din("wih1T", (640, G3), BF16)       # padded transposed w_ih1
    whh1T = din("whh1T", (H, G3), BF16)
    wih2T = din("wih2T", (H, G3), BF16)
    whh2T = din("whh2T", (H, G3), BF16)
    wih3T = din("wih3T", (H, G3), BF16)
    whh3T = din("whh3T", (H, G3), BF16)
    wl1T = din("wl1T", (H, F_IN), BF16)
    wl2T = din("wl2T", (H, F_IN), BF16)
    bih1T = din("bih1T", (128, 12), F32)        # b_ih1 chunk-transposed
    brows_r = din("brows_r", (3, H), F32)
    brows_z = din("brows_z", (3, H), F32)
    brows_gn = din("brows_gn", (3, H), F32)
    brows_in = din("brows_in", (3, H), F32)
    bsel = din("bsel", (3, 68), F32)
    bl = din("bl", (2, F_IN), F32)
    ones1 = din("ones1", (1, 128), F32)
    ident_bf = din("ident_bf", (128, 128), BF16)
    ident_f32 = din("ident_f32", (128, 128), F32)

    o1 = nc.dram_tensor("o1", [B_LOC, T, F_IN], F32, kind="ExternalOutput").ap()
    o2 = nc.dram_tensor("o2", [B_LOC, T, F_IN], F32, kind="ExternalOutput").ap()

    xr = x.rearrange("b t f -> (t b) f")    # row = 4t + b
    o1r = o1.rearrange("b t f -> (t b) f")
    o2r = o2.rearrange("b t f -> (t b) f")

    wT_names = [whh1T, wih2T, whh2T, wih3T, whh3T]

    with tile.TileContext(nc) as tc, ExitStack() as ctx:
        P = ctx.enter_context  # shorthand

        # ---------------- persistent SBUF ----------------
        wp = P(tc.tile_pool(name="wp", bufs=1))
        wT = [wp.tile([128, G3], BF16, tag=f"wT{i}_{k}")
              for i in range(5) for k in range(4)]
        wT = [wT[i * 4:(i + 1) * 4] for i in range(5)]  # [matrix][k]
        wi1 = [wp.tile([128, G3], BF16, tag=f"wi1_{k}") for k in range(5)]
        xT = [wp.tile([128, NTB], BF16, tag=f"xT{k}") for k in range(5)]
        gi1T = [wp.tile([128, NTB], BF16, tag=f"gi1T{c}") for c in range(12)]
        h3T = [wp.tile([128, B_LOC * (T + 1)], BF16, tag=f"h3T{k}") for k in range(4)]
        wl = [[wp.tile([128, F_IN], BF16, tag=f"wl{h}_{k}") for k in range(4)]
              for h in range(2)]
        hTall = [[wp.tile([128, 12], BF16, tag=f"hTall{p}_{k}") for k in range(4)]
                 for p in range(2)]
        hstk = [wp.tile([128, H], F32, tag=f"hstk{p}") for p in range(2)]
        bih1_sb = wp.tile([128, 12], F32, tag="bih1")
        br_sb = {}
        for nm, ap_ in (("r", brows_r), ("z", brows_z), ("gn", brows_gn),
                        ("in", brows_in)):
            br_sb[nm] = wp.tile([3, H], F32, tag=f"br_{nm}")
            nc.sync.dma_start(br_sb[nm][:], ap_)
        bsel_sb = wp.tile([3, 68], F32, tag="bsel")
        bl_sb = wp.tile([2, F_IN], F32, tag="bl")
        ones1_sb = wp.tile([1, 128], F32, tag="ones1")
        idb_sb = wp.tile([128, 128], BF16, tag="idb")
        idf_sb = wp.tile([128, 128], F32, tag="idf")

        nc.sync.dma_start(bih1_sb[:], bih1T)
        nc.sync.dma_start(bsel_sb[:], bsel)
        nc.sync.dma_start(bl_sb[:], bl)
        nc.sync.dma_start(ones1_sb[:], ones1)
        nc.sync.dma_start(idb_sb[:], ident_bf)
        nc.sync.dma_start(idf_sb[:], ident_f32)
        for i in range(5):
            for k in range(4):
                nc.sync.dma_start(wT[i][k][:], wT_names[i][k * 128:(k + 1) * 128, :])
        for k in range(5):
            nc.sync.dma_start(wi1[k][:], wih1T[k * 128:(k + 1) * 128, :])
        for h, w_ap in ((0, wl1T), (1, wl2T)):
            for k in range(4):
                nc.sync.dma_start(wl[h][k][:], w_ap[k * 128:(k + 1) * 128, :])

        # zero-init state
        for p in range(2):
            nc.vector.memset(hstk[p][:], 0.0)
            for k in range(4):
                nc.vector.memset(hTall[p][k][:], 0.0)
        for k in range(4):
            nc.vector.memset(h3T[k][:, 0:B_LOC], 0.0)
        nc.vector.memset(xT[4][:], 0.0)  # row 0 overwritten below

        # ---------------- phase 0: build xT (transposed x, bf16) -------------
        with tc.tile_pool(name="p0", bufs=3) as p0, \
             tc.tile_pool(name="p0ps", bufs=2, space="PSUM") as p0ps:
            for i in range(NTB // 128):
                xs = p0.tile([128, F_IN], F32, tag="xs")
                nc.sync.dma_start(xs[:], xr[i * 128:(i + 1) * 128, :])
                xb = p0.tile([128, 640], BF16, tag="xb")
                nc.scalar.copy(xb[:, 0:F_IN], xs[:])
                for c in range(5):
                    w_ = 128 if c < 4 else 1
                    tp = p0ps.tile([128, 128], BF16, tag="tp0")
                    nc.tensor.transpose(tp[0:w_, 0:128], xb[:, c * 128:c * 128 + w_],
                                        idb_sb[:])
                    nc.scalar.copy(xT[c][0:w_, i * 128:(i + 1) * 128], tp[0:w_, 0:128])

        # ---------------- phase 1: gi1T = w_ih1 @ x_t (+b_ih1), transposed ---
        with tc.tile_pool(name="p1ps", bufs=4, space="PSUM") as p1ps:
            for m in range(12):
                for n in range(NTB // 512):
                    ps = p1ps.tile([128, 512], F32, tag="ps1")
                    for k in range(5):
                        nc.tensor.matmul(ps[:], wi1[k][:, m * 128:(m + 1) * 128],
                                         xT[k][:, n * 512:(n + 1) * 512],
                                         start=(k == 0), stop=(k == 4))
                    nc.scalar.activation(gi1T[m][:, n * 512:(n + 1) * 512], ps[:],
                                         AF.Identity, bias=bih1_sb[:, m:m + 1])

        # ---------------- phase 2: recurrence ----------------
        lp = P(tc.tile_pool(name="lp", bufs=2))
        rzp = P(tc.tile_pool(name="rzp", bufs=2, space="PSUM"))
        gnp = P(tc.tile_pool(name="gnp", bufs=2, space="PSUM"))
        inp = P(tc.tile_pool(name="inp", bufs=2, space="PSUM"))

        for s in range(T + 2):
            a1 = s <= T - 1
            a2 = 1 <= s <= T
            a3 = 2 <= s <= T + 1
            lo = 0 if a1 else (32 if a2 else 64)
            hi = 68 if a3 else (36 if a2 else 4)
            pv = (s - 1) % 2   # parity holding states from step s-1
            cu = s % 2
            t1, t2, t3 = s, s - 1, s - 2

            ps_rz = rzp.tile([128, 1024], F32, tag="ps_rz")
            ps_gn = gnp.tile([128, 512], F32, tag="ps_gn")
            ps_in = inp.tile([128, 512], F32, tag="ps_in")

            # bias matmuls open each accumulation region
            nc.tensor.matmul(ps_rz[lo:hi, 0:512], bsel_sb[:, lo:hi], br_sb["r"][:],
                             start=True, stop=False)
            nc.tensor.matmul(ps_rz[lo:hi, 512:1024], bsel_sb[:, lo:hi],
                             br_sb["z"][:], start=True, stop=False)
            nc.tensor.matmul(ps_gn[lo:hi, :], bsel_sb[:, lo:hi], br_sb["gn"][:],
                             start=True, stop=False)
            nc.tensor.matmul(ps_in[lo:hi, :], bsel_sb[:, lo:hi], br_sb["in"][:],
                             start=True, stop=False)

            mms = []  # (psum_slice, lhsT, rhs)
            if a1:
                # inject gi1 (incl. b_ih1) for layer 1: r,z -> ps_rz, n -> ps_in
                for c in range(12):
                    tgt = ps_rz if c < 8 else ps_in
                    off = (c % 8) * 128 if c < 8 else (c - 8) * 128
                    mms.append((tgt[0:4, off:off + 128],
                                gi1T[c][:, 4 * t1:4 * t1 + 4], idb_sb[:], (0, 0)))
                for k in range(4):
                    st = hTall[pv][k][:, 0:4]
                    mms.append((ps_rz[0:4, :], st, wT[0][k][:, 0:1024], (0, 0)))
                    mms.append((ps_gn[0:4, :], st, wT[0][k][:, 1024:1536], (0, 0)))
            if a2:
                for k in range(4):
                    s1_ = hTall[pv][k][:, 0:4]   # h1(s-1)
                    s2_ = hTall[pv][k][:, 4:8]   # h2(s-2)
                    mms.append((ps_rz[32:36, :], s1_, wT[1][k][:, 0:1024], (0, 32)))
                    mms.append((ps_rz[32:36, :], s2_, wT[2][k][:, 0:1024], (0, 32)))
                    mms.append((ps_in[32:36, :], s1_, wT[1][k][:, 1024:1536], (0, 32)))
                    mms.append((ps_gn[32:36, :], s2_, wT[2][k][:, 1024:1536], (0, 32)))
            if a3:
                for k in range(4):
                    s2_ = hTall[pv][k][:, 4:8]                    # h2(s-2)
                    s3_ = h3T[k][:, 4 * t3:4 * t3 + 4]            # h3(s-3)
                    mms.append((ps_rz[64:68, :], s2_, wT[3][k][:, 0:1024], (0, 64)))
                    mms.append((ps_rz[64:68, :], s3_, wT[4][k][:, 0:1024], (0, 64)))
                    mms.append((ps_in[64:68, :], s2_, wT[3][k][:, 1024:1536], (0, 64)))
                    mms.append((ps_gn[64:68, :], s3_, wT[4][k][:, 1024:1536], (0, 64)))
            # emit, flagging last write per psum tile
            last_idx = {}
            for idx, (dst, _, _, _) in enumerate(mms):
                last_idx[dst.tensor.name if hasattr(dst, 'tensor') else id(dst.ap if hasattr(dst, 'ap') else dst)] = idx
            # simpler: mark stop on final 3 mms (one per tile) after emission order
            for idx, (dst, lhsT, rhs, tpos) in enumerate(mms):
                nc.tensor.matmul(dst, lhsT, rhs, start=False,
                                 stop=(idx >= len(mms) - 1), tile_position=tpos,
                                 skip_group_check=True)

            # gates
            rzs = lp.tile([128, 1024], F32, tag="rzs")
            nc.scalar.activation(rzs[lo:hi, :], ps_rz[lo:hi, :], AF.Sigmoid)
            tmp = lp.tile([128, 512], F32, tag="tmp")
            nc.vector.tensor_mul(tmp[lo:hi, :], rzs[lo:hi, 0:512], ps_gn[lo:hi, :])
            npre = lp.tile([128, 512], F32, tag="npre")
            nc.vector.tensor_add(npre[lo:hi, :], tmp[lo:hi, :], ps_in[lo:hi, :])
            nsb = lp.tile([128, 512], F32, tag="nsb")
            nc.scalar.activation(nsb[lo:hi, :], npre[lo:hi, :], AF.Tanh)
            hmn = lp.tile([128, 512], F32, tag="hmn")
            nc.vector.tensor_sub(hmn[lo:hi, :], hstk[pv][lo:hi, :], nsb[lo:hi, :])
            zm = lp.tile([128, 512], F32, tag="zm")
            nc.vector.tensor_mul(zm[lo:hi, :], rzs[lo:hi, 512:1024], hmn[lo:hi, :])
            nc.vector.tensor_add(hstk[cu][lo:hi, :], nsb[lo:hi, :], zm[lo:hi, :])

            # transpose h' -> hTall[cu] and h3 history
            if s <= T:
                for k in range(4):
                    tp = gnp.tile([128, 512], F32, tag="ps_gn")  # share gn bank
                    nc.tensor.transpose(tp[0:128, 0:68],
                                        hstk[cu][0:68, k * 128:(k + 1) * 128],
                                        idf_sb[0:68, 0:68])
                    nc.scalar.copy(hTall[cu][k][:, 0:4], tp[:, 0:4])
                    nc.scalar.copy(hTall[cu][k][:, 4:8], tp[:, 32:36])
                    if a3:
                        nc.vector.tensor_copy(
                            h3T[k][:, 4 * (t3 + 1):4 * (t3 + 2)], tp[:, 64:68])
            elif a3:  # s == T+1: still need h3(t3) for heads
                for k in range(4):
                    tp = gnp.tile([128, 512], F32, tag="ps_gn")
                    nc.tensor.transpose(tp[0:128, 64:68],
                                        hstk[cu][64:68, k * 128:(k + 1) * 128],
                                        idf_sb[64:68, 64:68])
                    nc.vector.tensor_copy(
                        h3T[k][:, 4 * (t3 + 1):4 * (t3 + 2)], tp[:, 64:68])

        # ---------------- phase 3: heads + masks ----------------
        with tc.tile_pool(name="p3", bufs=3) as p3, \
             tc.tile_pool(name="p3ps", bufs=4, space="PSUM") as p3ps:
            for i in range(NTB // 128):
                xs = p3.tile([128, F_IN], F32, tag="x3")
                nc.sync.dma_start(xs[:], xr[i * 128:(i + 1) * 128, :])
                ssb = []
                for h in range(2):
                    ps = p3ps.tile([128, 512], F32, tag="ps3")
                    psb = p3ps.tile([128, 8], F32, tag="ps3b")
                    for k in range(4):
                        st = h3T[k][:, B_LOC + 128 * i: B_LOC + 128 * (i + 1)]
                        nc.tensor.matmul(ps[:], st, wl[h][k][:, 0:512],
                                         start=(k == 0), stop=False)
                        nc.tensor.matmul(psb[:, 0:1], st, wl[h][k][:, 512:513],
                                         start=(k == 0), stop=False)
                    nc.tensor.matmul(ps[:], ones1_sb[:], bl_sb[h:h + 1, 0:512],
                                     start=False, stop=True)
                    nc.tensor.matmul(psb[:, 0:1], ones1_sb[:],
                                     bl_sb[h:h + 1, 512:513],
                                     start=False, stop=True)
                    sb = p3.tile([128, F_IN], F32, tag=f"s{h}")
                    nc.scalar.activation(sb[:, 0:512], ps[:], AF.Relu)
                    nc.scalar.activation(sb[:, 512:513], psb[:, 0:1], AF.Relu)
                    ssb.append(sb)
                den = p3.tile([128, F_IN], F32, tag="den")
                nc.vector.tensor_add(den[:], ssb[0][:], ssb[1][:])
                nc.vector.tensor_scalar_add(den[:], den[:], 1e-16)
                rec = p3.tile([128, F_IN], F32, tag="rec")
                nc.vector.reciprocal(rec[:], den[:])
                rx = p3.tile([128, F_IN], F32, tag="rx")
                nc.vector.tensor_mul(rx[:], rec[:], xs[:])
                for h, outr in ((0, o1r), (1, o2r)):
                    ob = p3.tile([128, F_IN], F32, tag=f"ob{h}")
                    nc.vector.tensor_mul(ob[:], ssb[h][:], rx[:])
                    nc.sync.dma_start(outr[i * 128:(i + 1) * 128, :], ob[:])

    _split_multi_waits(nc)
    return nc


_CACHE = {}


def _get_nc(T):
    if T not in _CACHE:
        _CACHE[T] = _build(T)
    return _CACHE[T]


def _prep_shared(weights, T):
    w = {k: np.asarray(v, dtype=np.float32) for k, v in weights.items()}
    bf = ml_dtypes.bfloat16
    out = {}
    wih1T = np.zeros((640, G3), np.float32)
    wih1T[:F_IN, :] = w["w_ih1"].T
    out["wih1T"] = wih1T.astype(bf)
    for nm, key in (("whh1T", "w_hh1"), ("wih2T", "w_ih2"), ("whh2T", "w_hh2"),
                    ("wih3T", "w_ih3"), ("whh3T", "w_hh3")):
        out[nm] = np.ascontiguousarray(w[key].T).astype(bf)
    out["wl1T"] = np.ascontiguousarray(w["w_l1"].T).astype(bf)
    out["wl2T"] = np.ascontiguousarray(w["w_l2"].T).astype(bf)
    out["bih1T"] = np.ascontiguousarray(w["b_ih1"].reshape(12, 128).T).astype(np.float32)
    b_hh1, b_hh2, b_hh3 = w["b_hh1"], w["b_hh2"], w["b_hh3"]
    b_ih2, b_ih3 = w["b_ih2"], w["b_ih3"]
    out["brows_r"] = np.stack([b_hh1[0:H], b_ih2[0:H] + b_hh2[0:H],
                               b_ih3[0:H] + b_hh3[0:H]]).astype(np.float32)
    out["brows_z"] = np.stack([b_hh1[H:2*H], b_ih2[H:2*H] + b_hh2[H:2*H],
                               b_ih3[H:2*H] + b_hh3[H:2*H]]).astype(np.float32)
    out["brows_gn"] = np.stack([b_hh1[2*H:], b_hh2[2*H:], b_hh3[2*H:]]).astype(np.float32)
    out["brows_in"] = np.stack([np.zeros(H, np.float32), b_ih2[2*H:],
                                b_ih3[2*H:]]).astype(np.float32)
    bsel = np.zeros((3, 68), np.float32)
    for l_ in range(3):
        bsel[l_, 32 * l_:32 * l_ + B_LOC] = 1.0
    out["bsel"] = bsel
    out["bl"] = np.stack([w["b_l1"], w["b_l2"]]).astype(np.float32)
    out["ones1"] = np.ones((1, 128), np.float32)
    out["ident_bf"] = np.eye(128, dtype=np.float32).astype(bf)
    out["ident_f32"] = np.eye(128, dtype=np.float32)
    return out


def kernel(**inputs):
    x_full = np.asarray(inputs["x"], dtype=np.float32)
    Bx, T, F = x_full.shape
    assert (Bx, F) == (B, F_IN)
    weights = {k: v for k, v in inputs.items() if k != "x"}
    nc = _get_nc(T)
    shared = _prep_shared(weights, T)
    in_maps = []
    for c in range(N_CORES):
        m = dict(shared)
        m["x"] = np.ascontiguousarray(x_full[c * B_LOC:(c + 1) * B_LOC])
        in_maps.append(m)
    res = run_bass_kernel_spmd(nc, in_maps, list(range(N_CORES)))
    o1 = np.concatenate([r["o1"] for r in res.results], axis=0)
    o2 = np.concatenate([r["o2"] for r in res.results], axis=0)
    return o1, o2


# revision 24
# speedup vs baseline: 1.0342x; 1.0342x over previous
"""Trainium2 Bass kernel for 3-layer GRU + dual mask heads.

Model (eval): x [32, 512, 513] -> 3x GRUCell(H=512) scan over T -> two linear
heads (513 out) + relu -> normalized masks -> (mask1*x, mask2*x).

Strategy: data-parallel over batch (4 per core, 8 cores). Per core, the
recurrence runs as a layer-wavefront (layer l processes time t=s-l at step s) so
every matmul at a step depends only on states from step s-1. States are kept
transposed ([H, 4] bf16) and used as matmul stationaries; weights stream as the
moving operand. Gate elementwise math runs on stacked [row-block, 512] tiles
(L1 rows 0-3, L2 32-35, L3 64-67 via PE column tiling). gi1 = W_ih1 @ x_t for
all t is precomputed with big matmuls into transposed layout and injected into
the per-step PSUM accumulation through identity matmuls. h' is re-transposed
each step with PE transposes. Heads run as big matmuls at the end.
"""
import sys
import numpy as np

sys.path.insert(0, "/opt/trn_rl_repo")

import ml_dtypes  # noqa: E402
from contextlib import ExitStack  # noqa: E402

import concourse.bass as bass  # noqa: E402
import concourse.tile as tile  # noqa: E402
import bass_rust  # noqa: E402
from concourse import mybir  # noqa: E402
from concourse.bass_utils import run_bass_kernel_spmd  # noqa: E402
from bass_rust import add_dep_helper  # noqa: E402

B, T_FULL, F_IN, H = 32, 512, 513, 512
G3 = 3 * H  # 1536
N_CORES = 8
B_LOC = B // N_CORES  # 4
F32 = mybir.dt.float32
BF16 = mybir.dt.bfloat16
AF = mybir.ActivationFunctionType

_SPLIT_CNT = [0]


def _split_multi_waits(nc):
    """This walrus build supports only ONE sem-wait per instruction; split
    extra on_wait entries into preceding single-wait NoOps on the same engine."""
    total = 0
    for fn in nc.m.functions:
        for bb in fn.blocks:
            out = []
            changed = False
            for inst in bb.instructions:
                si = getattr(inst, "sync_info", None)
                ow = si.on_wait if si is not None else None
                if ow and len(ow) > 1:
                    extra = list(ow[:-1])
                    del ow[:-1]
                    for w in extra:
                        _SPLIT_CNT[0] += 1
                        total += 1
                        nd = mybir.InstNoOp(
                            name=f"I-wsplit-{_SPLIT_CNT[0]}", ins=[], outs=[],
                            engine=inst.engine,
                        )
                        nd.sync_info = bass_rust.SyncInfo(on_wait=[w], on_update=[])
                        out.append(nd)
                    changed = True
                out.append(inst)
            if changed:
                bb.instructions[:] = out
    return total


def _build(T, debug=False):
    assert T % 32 == 0
    NTB = B_LOC * T  # rows (t-major: row = 4t+b)
    nc = bass.Bass("TRN2", target_bir_lowering=False, debug=False,
                   num_devices=N_CORES)

    dt_in = {}

    def din(name, shape, dt):
        dt_in[name] = (shape, dt)
        return nc.dram_tensor(name, list(shape), dt, kind="ExternalInput").ap()

    x = din("x", (B_LOC * T, F_IN), F32)  # host passes t-major rows (4t+b)
    wih1T = din("wih1T", (640, G3), BF16)       # padded transposed w_ih1
    whh1T = din("whh1T", (H, G3), BF16)
    wih2T = din("wih2T", (H, G3), BF16)
    whh2T = din("whh2T", (H, G3), BF16)
    wih3T = din("wih3T", (H, G3), BF16)
    whh3T = din("whh3T", (H, G3), BF16)
    wl1T = din("wl1T", (H, F_IN), F32)
    wl2T = din("wl2T", (H, F_IN), F32)
    bih1T = din("bih1T", (128, 12), F32)        # b_ih1 chunk-transposed
    brflat = din("brflat", (65, 3 * H), F32)
    onesb = din("onesb", (65, 4), F32)
    bl1 = din("bl1", (1, F_IN), F32)
    bl2 = din("bl2", (1, F_IN), F32)
    ones1 = din("ones1", (1, 128), F32)
    ident_bf = din("ident_bf", (128, 128), BF16)
    ident_f32 = din("ident_f32", (128, 128), F32)

    o1 = nc.dram_tensor("o1", [B_LOC * T, F_IN], F32, kind="ExternalOutput").ap()
    o2 = nc.dram_tensor("o2", [B_LOC * T, F_IN], F32, kind="ExternalOutput").ap()
    if debug:
        dbg_gi = nc.dram_tensor("dbg_gi", [12 * 128, B_LOC * T], F32,
                                kind="ExternalOutput").ap()
        dbg_h3 = nc.dram_tensor("dbg_h3", [4 * 128, B_LOC * T], F32,
                                kind="ExternalOutput").ap()

    xr, o1r, o2r = x, o1, o2

    wT_names = [whh1T, wih2T, whh2T, wih3T, whh3T]

    with tile.TileContext(nc) as tc, ExitStack() as ctx:
        P = ctx.enter_context  # shorthand

        # ---------------- persistent SBUF ----------------
        wp = P(tc.tile_pool(name="wp", bufs=1))
        loopres_ctx = ExitStack()
        lrp = loopres_ctx.enter_context(tc.tile_pool(name="lrp", bufs=1))
        wT = [lrp.tile([128, G3], BF16, name=f"wT{i}_{k}", tag=f"wT{i}_{k}")
              for i in range(5) for k in range(4)]
        wT = [wT[i * 4:(i + 1) * 4] for i in range(5)]  # [matrix][k]
        gi1T = [lrp.tile([128, NTB], BF16, name=f"gi1T{c}", tag=f"gi1T{c}") for c in range(12)]
        h3T = [wp.tile([128, B_LOC * T], F32, name=f"h3T{k}", tag=f"h3T{k}") for k in range(4)]
        hstk = [wp.tile([128, H], F32, name=f"hstk{p}", tag=f"hstk{p}") for p in range(2)]
        bih1_sb = wp.tile([128, 12], F32, tag="bih1")
        brf_sb = wp.tile([65, 3 * H], F32, tag="brf")
        onesb_sb = wp.tile([65, 4], F32, tag="onesb")

        ones1_sb = wp.tile([1, 128], F32, tag="ones1")
        idb_sb = wp.tile([128, 128], BF16, tag="idb")
        idf_sb = wp.tile([128, 128], F32, tag="idf")

        nc.sync.dma_start(bih1_sb[:], bih1T)
        nc.sync.dma_start(brf_sb[:], brflat)
        nc.sync.dma_start(onesb_sb[:], onesb)
        nc.sync.dma_start(ones1_sb[:], ones1)
        nc.sync.dma_start(idb_sb[:], ident_bf)
        nc.sync.dma_start(idf_sb[:], ident_f32)
        for i in range(5):
            for k in range(4):
                nc.sync.dma_start(wT[i][k][:], wT_names[i][k * 128:(k + 1) * 128, :])

        # zero-init state
        for p in range(2):
            nc.vector.memset(hstk[p][:], 0.0)
        # ---------------- phases 0+1 (transient xT/wi1 pool) -----------------
        xp_ctx = ExitStack()
        xp = xp_ctx.enter_context(tc.tile_pool(name="xp", bufs=1))
        wi1 = [xp.tile([128, G3], BF16, name=f"wi1_{k}", tag=f"wi1_{k}") for k in range(5)]
        xT = [xp.tile([128, NTB], BF16, name=f"xT{k}", tag=f"xT{k}") for k in range(5)]
        for k in range(5):
            nc.sync.dma_start(wi1[k][:], wih1T[k * 128:(k + 1) * 128, :])
        nc.vector.memset(xT[4][:], 0.0)  # row 0 overwritten below

        # ---------------- phase 0: build xT (transposed x, bf16) -------------
        with tc.tile_pool(name="p0", bufs=3) as p0, \
             tc.tile_pool(name="p0ps", bufs=2, space="PSUM") as p0ps:
            for i in range(NTB // 128):
                xs = p0.tile([128, F_IN], F32, tag="xs")
                nc.sync.dma_start(xs[:], xr[i * 128:(i + 1) * 128, :])
                xb = p0.tile([128, 640], BF16, tag="xb")
                nc.scalar.copy(xb[:, 0:F_IN], xs[:])
                for c in range(5):
                    w_ = 128 if c < 4 else 1
                    tp = p0ps.tile([128, 128], BF16, tag="tp0")
                    nc.tensor.transpose(tp[0:w_, 0:128], xb[:, c * 128:c * 128 + w_],
                                        idb_sb[:])
                    nc.scalar.copy(xT[c][0:w_, i * 128:(i + 1) * 128], tp[0:w_, 0:128])

        # ---------------- phase 1: gi1T = w_ih1 @ x_t (+b_ih1), transposed ---
        with tc.tile_pool(name="p1ps", bufs=4, space="PSUM") as p1ps:
            for m in range(12):
                for n0 in range(0, NTB, 512):
                    w_ = min(512, NTB - n0)
                    ps = p1ps.tile([128, 512], F32, tag="ps1")
                    for k in range(5):
                        nc.tensor.matmul(ps[:, 0:w_], wi1[k][:, m * 128:(m + 1) * 128],
                                         xT[k][:, n0:n0 + w_],
                                         start=(k == 0), stop=(k == 4))
                    nc.scalar.activation(gi1T[m][:, n0:n0 + w_], ps[:, 0:w_],
                                         AF.Identity, bias=bih1_sb[:, m:m + 1])

        xp_ctx.close()

        # ---------------- phase 2: recurrence ----------------
        # Skew-2 wavefront: layer l processes t = s - 2l at step s, so the
        # cross-layer matmul inputs (gi2/gi3) come from TWO steps back and can
        # overlap the previous step's gate chain; only the self-recurrent gh
        # matmuls wait on the 1-back chain.
        loop_ctx = ExitStack()
        lp = loop_ctx.enter_context(tc.tile_pool(name="lp", bufs=2))
        psp = loop_ctx.enter_context(tc.tile_pool(name="psp", bufs=1, space="PSUM"))
        prz = [psp.tile([128, 1024], F32, name=f"prz{p}", tag=f"prz{p}")
               for p in range(2)]
        pgn = [psp.tile([128, 512], F32, name=f"pgn{p}", tag=f"pgn{p}")
               for p in range(2)]
        pin_ = psp.tile([128, 512], F32, name="pin", tag="pin")
        ptp = psp.tile([128, 512], F32, name="ptp", tag="ptp")
        for p in range(2):
            nc.vector.memset(prz[p][:], 0.0)
            nc.vector.memset(pgn[p][:], 0.0)
        nc.vector.memset(pin_[:], 0.0)
        nc.vector.memset(ptp[:], 0.0)
        hT3 = [[wp.tile([128, 68], BF16, name=f"hT3_{p}_{k}", tag=f"hT3_{p}_{k}")
                for k in range(4)] for p in range(3)]
        for p in range(3):
            for k in range(4):
                nc.vector.memset(hT3[p][k][:], 0.0)

        for s in range(T + 4):
            a1 = s <= T - 1
            a2 = 2 <= s <= T + 1
            a3 = 4 <= s <= T + 3
            b1m = (s - 1) % 3   # states written at s-1 (gh inputs)
            b2m = (s - 2) % 3   # states written at s-2 (gi inputs)
            pv = (s - 1) % 2
            cu = s % 2
            t1, t3 = s, s - 4

            ps_rz = prz[s % 2]
            ps_gn = pgn[s % 2]
            ps_in = pin_

            mms = []  # (region_key, dst, lhsT, rhs, tile_position, pri)
            # pri 0: no-dep / 2-back inputs first; pri 1: 1-back (gh) inputs
            if a1:
                for c in range(12):
                    if c < 8:
                        key, dst = ("rz", 0, (c // 4) * 512), ps_rz[0:4, c * 128:(c + 1) * 128]
                    else:
                        key, dst = ("in", 0, (c - 8) * 128), ps_in[0:4, (c - 8) * 128:(c - 7) * 128]
                    mms.append((key, dst, gi1T[c][:, 4 * t1:4 * t1 + 4],
                                idb_sb[:], (0, 0), 0))
                for k in range(4):
                    st = hT3[b1m][k][:, 0:4]
                    mms.append((("rz", 0, 0), ps_rz[0:4, 0:512], st,
                                wT[0][k][:, 0:512], (0, 0), 1))
                    mms.append((("rz", 0, 512), ps_rz[0:4, 512:1024], st,
                                wT[0][k][:, 512:1024], (0, 0), 1))
                    mms.append((("gn", 0, 0), ps_gn[0:4, :], st,
                                wT[0][k][:, 1024:1536], (0, 0), 1))
            if a2:
                for k in range(4):
                    s1_ = hT3[b2m][k][:, 0:4]    # h1(s-2)
                    s2_ = hT3[b1m][k][:, 32:36]  # h2(s-3)
                    mms.append((("rz", 32, 0), ps_rz[32:36, 0:512], s1_,
                                wT[1][k][:, 0:512], (0, 32), 0))
                    mms.append((("rz", 32, 512), ps_rz[32:36, 512:1024], s1_,
                                wT[1][k][:, 512:1024], (0, 32), 0))
                    mms.append((("in", 32, 0), ps_in[32:36, :], s1_,
                                wT[1][k][:, 1024:1536], (0, 32), 0))
                    mms.append((("rz", 32, 0), ps_rz[32:36, 0:512], s2_,
                                wT[2][k][:, 0:512], (0, 32), 1))
                    mms.append((("rz", 32, 512), ps_rz[32:36, 512:1024], s2_,
                                wT[2][k][:, 512:1024], (0, 32), 1))
                    mms.append((("gn", 32, 0), ps_gn[32:36, :], s2_,
                                wT[2][k][:, 1024:1536], (0, 32), 1))
            if a3:
                for k in range(4):
                    s2_ = hT3[b2m][k][:, 32:36]  # h2(s-4)
                    s3_ = hT3[b1m][k][:, 64:68]  # h3(s-5)
                    mms.append((("rz", 64, 0), ps_rz[64:68, 0:512], s2_,
                                wT[3][k][:, 0:512], (0, 64), 0))
                    mms.append((("rz", 64, 512), ps_rz[64:68, 512:1024], s2_,
                                wT[3][k][:, 512:1024], (0, 64), 0))
                    mms.append((("in", 64, 0), ps_in[64:68, :], s2_,
                                wT[3][k][:, 1024:1536], (0, 64), 0))
                    mms.append((("rz", 64, 0), ps_rz[64:68, 0:512], s3_,
                                wT[4][k][:, 0:512], (0, 64), 1))
                    mms.append((("rz", 64, 512), ps_rz[64:68, 512:1024], s3_,
                                wT[4][k][:, 512:1024], (0, 64), 1))
                    mms.append((("gn", 64, 0), ps_gn[64:68, :], s3_,
                                wT[4][k][:, 1024:1536], (0, 64), 1))
            bia = []
            if a1:
                bia.append((("gn", 0, 0), ps_gn[0:4, :], 0, (0, 0)))
            if a2:
                bia += [(("rz", 32, 0), ps_rz[32:36, 0:512], 1, (0, 32)),
                        (("rz", 32, 512), ps_rz[32:36, 512:1024], 2, (0, 32)),
                        (("gn", 32, 0), ps_gn[32:36, :], 3, (0, 32)),
                        (("in", 32, 0), ps_in[32:36, :], 4, (0, 32))]
            if a3:
                bia += [(("rz", 64, 0), ps_rz[64:68, 0:512], 5, (0, 64)),
                        (("rz", 64, 512), ps_rz[64:68, 512:1024], 6, (0, 64)),
                        (("gn", 64, 0), ps_gn[64:68, :], 7, (0, 64)),
                        (("in", 64, 0), ps_in[64:68, :], 8, (0, 64))]
            for key, dst, bi, tpos in bia:
                r_ = 32 * (bi // 3)
                c_ = bi % 3
                mms.append((key, dst, onesb_sb[r_:r_ + 1, 0:4],
                            brf_sb[r_:r_ + 1, c_ * H:(c_ + 1) * H],
                            (r_, tpos[1]), 0))
            # round-robin merge across col-group lanes, 2-back work first
            merged = []
            for pri in (0, 1):
                queues = {0: [], 1: [], 2: []}
                for m_ in mms:
                    if m_[5] == pri:
                        queues[m_[4][1] // 32].append(m_)
                qi = {g: 0 for g in queues}
                while True:
                    emitted = False
                    for g in (0, 1, 2):
                        if qi[g] < len(queues[g]):
                            merged.append(queues[g][qi[g]])
                            qi[g] += 1
                            emitted = True
                    if not emitted:
                        break
            started = set()
            last = {}
            for idx, (key, *_r) in enumerate(merged):
                last[key] = idx
            prev_mm = None
            for idx, (key, dst, lhsT, rhs, tpos, _pri) in enumerate(merged):
                st_flag = key not in started
                started.add(key)
                mi = nc.tensor.matmul(dst, lhsT, rhs, start=st_flag,
                                      stop=(last[key] == idx),
                                      tile_position=tpos, skip_group_check=True)
                # pin the lane-interleaved PE order past the Tile scheduler
                if prev_mm is not None:
                    add_dep_helper(mi.ins, prev_mm, sync=False,
                                   reason="lane-interleave order")
                prev_mm = mi.ins

            # gates (on [0:68]; inactive rows compute garbage that is unused)
            r_sb = lp.tile([128, 512], F32, tag="r_sb")
            nc.scalar.activation(r_sb[0:68, :], ps_rz[0:68, 0:512], AF.Sigmoid)
            z_sb = lp.tile([128, 512], F32, tag="z_sb")
            nc.scalar.activation(z_sb[0:68, :], ps_rz[0:68, 512:1024], AF.Sigmoid)
            omz = lp.tile([128, 512], F32, tag="omz")
            nc.vector.tensor_scalar(omz[0:68, :], z_sb[0:68, :], -1.0, 1.0,
                                    mybir.AluOpType.mult, mybir.AluOpType.add)
            zh = lp.tile([128, 512], F32, tag="zh")
            nc.vector.tensor_mul(zh[0:68, :], z_sb[0:68, :], hstk[pv][0:68, :])
            tmp = lp.tile([128, 512], F32, tag="tmp")
            nc.vector.tensor_mul(tmp[0:68, :], r_sb[0:68, :], ps_gn[0:68, :])
            npre = lp.tile([128, 512], F32, tag="npre")
            nc.vector.tensor_add(npre[0:68, :], tmp[0:68, :], ps_in[0:68, :])
            nsb = lp.tile([128, 512], F32, tag="nsb")
            nc.scalar.activation(nsb[0:68, :], npre[0:68, :], AF.Tanh)
            t1s = lp.tile([128, 512], F32, tag="t1s")
            nc.vector.tensor_mul(t1s[0:68, :], omz[0:68, :], nsb[0:68, :])
            nc.vector.tensor_add(hstk[cu][0:68, :], t1s[0:68, :], zh[0:68, :])

            # transpose h' chunks and store transposed states
            if s <= T + 2:
                tp = ptp
                for k in range(4):
                    nc.tensor.transpose(tp[0:128, k * 128:k * 128 + 68],
                                        hstk[cu][0:68, k * 128:(k + 1) * 128],
                                        idf_sb[0:68, 0:68])
                for k in range(4):
                    nc.scalar.copy(hT3[s % 3][k][:], tp[:, k * 128:k * 128 + 68])
                    if a3:
                        nc.vector.tensor_copy(h3T[k][:, 4 * t3:4 * t3 + 4],
                                              tp[:, k * 128 + 64:k * 128 + 68])
            elif a3:  # s == T+3
                tp = ptp
                for k in range(4):
                    nc.tensor.transpose(tp[0:128, k * 128 + 64:k * 128 + 68],
                                        hstk[cu][64:68, k * 128:(k + 1) * 128],
                                        idf_sb[64:68, 64:68])
                for k in range(4):
                    nc.vector.tensor_copy(h3T[k][:, 4 * t3:4 * t3 + 4],
                                          tp[:, k * 128 + 64:k * 128 + 68])

        loop_ctx.close()
        if not debug:
            loopres_ctx.close()

        if debug:
            with tc.tile_pool(name="dbgp", bufs=2) as dbgp:
                for c in range(12):
                    t_ = dbgp.tile([128, B_LOC * T], F32, tag="dbg_t")
                    nc.scalar.copy(t_[:], gi1T[c][:])
                    nc.sync.dma_start(dbg_gi[c * 128:(c + 1) * 128, :], t_[:])
                for k in range(4):
                    t_ = dbgp.tile([128, B_LOC * T], F32, tag="dbg_t2")
                    nc.scalar.copy(t_[:], h3T[k][:])
                    nc.sync.dma_start(dbg_h3[k * 128:(k + 1) * 128, :], t_[:])
            loopres_ctx.close()

        # ---------------- phase 3: heads + masks ----------------
        with tc.tile_pool(name="p3", bufs=3) as p3, \
             tc.tile_pool(name="p3ps", bufs=4, space="PSUM") as p3ps:
            wl = [[p3.tile([128, F_IN], F32, name=f"wl{h}_{k}", tag=f"wl{h}_{k}",
                           bufs=1) for k in range(4)] for h in range(2)]
            bl_sb = [p3.tile([1, F_IN], F32, name=f"bl_{h}", tag=f"bl_{h}", bufs=1)
                     for h in range(2)]
            nc.sync.dma_start(bl_sb[0][:], bl1)
            nc.sync.dma_start(bl_sb[1][:], bl2)
            for h, w_ap in ((0, wl1T), (1, wl2T)):
                for k in range(4):
                    nc.sync.dma_start(wl[h][k][:], w_ap[k * 128:(k + 1) * 128, :])
            for i in range(NTB // 128):
                xs = p3.tile([128, F_IN], F32, tag="x3")
                nc.sync.dma_start(xs[:], xr[i * 128:(i + 1) * 128, :])
                ssb = []
                for h in range(2):
                    ps = p3ps.tile([128, 512], F32, tag="ps3")
                    psb = p3ps.tile([128, 8], F32, tag="ps3b")
                    for k in range(4):
                        st = h3T[k][:, 128 * i: 128 * (i + 1)]
                        nc.tensor.matmul(ps[:], st, wl[h][k][:, 0:512],
                                         start=(k == 0), stop=False)
                        nc.tensor.matmul(psb[:, 0:1], st, wl[h][k][:, 512:513],
                                         start=(k == 0), stop=False)
                    nc.tensor.matmul(ps[:], ones1_sb[:], bl_sb[h][:, 0:512],
                                     start=False, stop=True)
                    nc.tensor.matmul(psb[:, 0:1], ones1_sb[:],
                                     bl_sb[h][:, 512:513],
                                     start=False, stop=True)
                    sb = p3.tile([128, F_IN], F32, tag=f"s{h}")
                    nc.scalar.activation(sb[:, 0:512], ps[:], AF.Relu)
                    nc.scalar.activation(sb[:, 512:513], psb[:, 0:1], AF.Relu)
                    ssb.append(sb)
                den = p3.tile([128, F_IN], F32, tag="den")
                nc.vector.tensor_add(den[:], ssb[0][:], ssb[1][:])
                nc.vector.tensor_scalar_add(den[:], den[:], 1e-16)
                rec = p3.tile([128, F_IN], F32, tag="rec")
                nc.vector.reciprocal(rec[:], den[:])
                rx = p3.tile([128, F_IN], F32, tag="rx")
                nc.vector.tensor_mul(rx[:], rec[:], xs[:])
                for h, outr in ((0, o1r), (1, o2r)):
                    ob = p3.tile([128, F_IN], F32, tag=f"ob{h}")
                    nc.vector.tensor_mul(ob[:], ssb[h][:], rx[:])
                    nc.sync.dma_start(outr[i * 128:(i + 1) * 128, :], ob[:])

    return nc


_CACHE = {}


def _get_nc(T, debug=False):
    key = (T, debug)
    if key not in _CACHE:
        _CACHE[key] = _build(T, debug)
    return _CACHE[key]


def _get_nc_split(T):
    """nc with the walrus single-wait fixup applied (hardware path)."""
    nc = _get_nc(T)
    if not getattr(nc, "_waits_split", False):
        _split_multi_waits(nc)
        nc._waits_split = True
    return nc


def _prep_shared(weights, T):
    w = {k: np.asarray(v, dtype=np.float32) for k, v in weights.items()}
    bf = ml_dtypes.bfloat16
    out = {}
    wih1T = np.zeros((640, G3), np.float32)
    wih1T[:F_IN, :] = w["w_ih1"].T
    out["wih1T"] = wih1T.astype(bf)
    for nm, key in (("whh1T", "w_hh1"), ("wih2T", "w_ih2"), ("whh2T", "w_hh2"),
                    ("wih3T", "w_ih3"), ("whh3T", "w_hh3")):
        out[nm] = np.ascontiguousarray(w[key].T).astype(bf)
    out["wl1T"] = np.ascontiguousarray(w["w_l1"].T).astype(np.float32)
    out["wl2T"] = np.ascontiguousarray(w["w_l2"].T).astype(np.float32)
    comb = w["b_ih1"].copy()
    comb[0:2 * H] += w["b_hh1"][0:2 * H]   # L1 rz bias rides the gi1 injection
    out["bih1T"] = np.ascontiguousarray(comb.reshape(12, 128).T).astype(np.float32)
    b_hh1, b_hh2, b_hh3 = w["b_hh1"], w["b_hh2"], w["b_hh3"]
    b_ih2, b_ih3 = w["b_ih2"], w["b_ih3"]
    rows = [
        b_hh1[2*H:],                                   # (L1, gn)
        b_ih2[0:H] + b_hh2[0:H], b_ih2[H:2*H] + b_hh2[H:2*H],
        b_hh2[2*H:], b_ih2[2*H:],                      # L2: r, z, gn, in
        b_ih3[0:H] + b_hh3[0:H], b_ih3[H:2*H] + b_hh3[H:2*H],
        b_hh3[2*H:], b_ih3[2*H:],                      # L3: r, z, gn, in
    ]
    brf = np.zeros((65, 3 * H), np.float32)
    for bi, row in enumerate(rows):
        brf[32 * (bi // 3), (bi % 3) * H:(bi % 3 + 1) * H] = row
    out["brflat"] = brf
    onesb = np.zeros((65, 4), np.float32)
    onesb[[0, 32, 64], :] = 1.0
    out["onesb"] = onesb
    out["bl1"] = w["b_l1"].reshape(1, -1).astype(np.float32)
    out["bl2"] = w["b_l2"].reshape(1, -1).astype(np.float32)
    out["ones1"] = np.ones((1, 128), np.float32)
    out["ident_bf"] = np.eye(128, dtype=np.float32).astype(bf)
    out["ident_f32"] = np.eye(128, dtype=np.float32)
    return out


def kernel(**inputs):
    x_full = np.asarray(inputs["x"], dtype=np.float32)
    Bx, T, F = x_full.shape
    assert (Bx, F) == (B, F_IN)
    weights = {k: v for k, v in inputs.items() if k != "x"}
    nc = _get_nc_split(T)
    shared = _prep_shared(weights, T)
    in_maps = []
    for c in range(N_CORES):
        m = dict(shared)
        xc = x_full[c * B_LOC:(c + 1) * B_LOC]            # [4, T, F]
        m["x"] = np.ascontiguousarray(xc.transpose(1, 0, 2).reshape(B_LOC * T, F_IN))
        in_maps.append(m)
    res = run_bass_kernel_spmd(nc, in_maps, list(range(N_CORES)))

    def unpack(name):
        parts = [r[name].reshape(T, B_LOC, F_IN).transpose(1, 0, 2)
                 for r in res.results]
        return np.concatenate(parts, axis=0)

    return unpack("o1"), unpack("o2")


# revision 25
# speedup vs baseline: 1.0482x; 1.0136x over previous
"""Trainium2 Bass kernel for 3-layer GRU + dual mask heads.

Model (eval): x [32, 512, 513] -> 3x GRUCell(H=512) scan over T -> two linear
heads (513 out) + relu -> normalized masks -> (mask1*x, mask2*x).

Strategy: data-parallel over batch (4 per core, 8 cores). Per core, the
recurrence runs as a layer-wavefront (layer l processes time t=s-l at step s) so
every matmul at a step depends only on states from step s-1. States are kept
transposed ([H, 4] bf16) and used as matmul stationaries; weights stream as the
moving operand. Gate elementwise math runs on stacked [row-block, 512] tiles
(L1 rows 0-3, L2 32-35, L3 64-67 via PE column tiling). gi1 = W_ih1 @ x_t for
all t is precomputed with big matmuls into transposed layout and injected into
the per-step PSUM accumulation through identity matmuls. h' is re-transposed
each step with PE transposes. Heads run as big matmuls at the end.
"""
import sys
import numpy as np

sys.path.insert(0, "/opt/trn_rl_repo")

import ml_dtypes  # noqa: E402
from contextlib import ExitStack  # noqa: E402

import concourse.bass as bass  # noqa: E402
import concourse.tile as tile  # noqa: E402
import bass_rust  # noqa: E402
from concourse import mybir  # noqa: E402
from concourse.bass_utils import run_bass_kernel_spmd  # noqa: E402
from bass_rust import add_dep_helper  # noqa: E402

B, T_FULL, F_IN, H = 32, 512, 513, 512
G3 = 3 * H  # 1536
N_CORES = 8
B_LOC = B // N_CORES  # 4
F32 = mybir.dt.float32
BF16 = mybir.dt.bfloat16
AF = mybir.ActivationFunctionType

_SPLIT_CNT = [0]


def _split_multi_waits(nc):
    """This walrus build supports only ONE sem-wait per instruction; split
    extra on_wait entries into preceding single-wait NoOps on the same engine."""
    total = 0
    for fn in nc.m.functions:
        for bb in fn.blocks:
            out = []
            changed = False
            for inst in bb.instructions:
                si = getattr(inst, "sync_info", None)
                ow = si.on_wait if si is not None else None
                if ow and len(ow) > 1:
                    extra = list(ow[:-1])
                    del ow[:-1]
                    for w in extra:
                        _SPLIT_CNT[0] += 1
                        total += 1
                        nd = mybir.InstNoOp(
                            name=f"I-wsplit-{_SPLIT_CNT[0]}", ins=[], outs=[],
                            engine=inst.engine,
                        )
                        nd.sync_info = bass_rust.SyncInfo(on_wait=[w], on_update=[])
                        out.append(nd)
                    changed = True
                out.append(inst)
            if changed:
                bb.instructions[:] = out
    return total


def _build(T, debug=False):
    assert T % 32 == 0
    NTB = B_LOC * T  # rows (t-major: row = 4t+b)
    nc = bass.Bass("TRN2", target_bir_lowering=False, debug=False,
                   num_devices=N_CORES)

    dt_in = {}

    def din(name, shape, dt):
        dt_in[name] = (shape, dt)
        return nc.dram_tensor(name, list(shape), dt, kind="ExternalInput").ap()

    x = din("x", (B_LOC * T, F_IN), F32)  # host passes t-major rows (4t+b)
    wih1T = din("wih1T", (640, G3), BF16)       # padded transposed w_ih1
    whh1T = din("whh1T", (H, G3), BF16)
    wih2T = din("wih2T", (H, G3), BF16)
    whh2T = din("whh2T", (H, G3), BF16)
    wih3T = din("wih3T", (H, G3), BF16)
    whh3T = din("whh3T", (H, G3), BF16)
    wl1T = din("wl1T", (H, F_IN), F32)
    wl2T = din("wl2T", (H, F_IN), F32)
    bih1T = din("bih1T", (128, 12), F32)        # b_ih1 chunk-transposed
    brflat = din("brflat", (65, 3 * H), F32)
    onesb = din("onesb", (65, 4), F32)
    bl1 = din("bl1", (1, F_IN), F32)
    bl2 = din("bl2", (1, F_IN), F32)
    ones1 = din("ones1", (1, 128), F32)
    ident_bf = din("ident_bf", (128, 128), BF16)
    ident_f32 = din("ident_f32", (128, 128), F32)

    o1 = nc.dram_tensor("o1", [B_LOC * T, F_IN], F32, kind="ExternalOutput").ap()
    o2 = nc.dram_tensor("o2", [B_LOC * T, F_IN], F32, kind="ExternalOutput").ap()
    if debug:
        dbg_gi = nc.dram_tensor("dbg_gi", [12 * 128, B_LOC * T], F32,
                                kind="ExternalOutput").ap()
        dbg_h3 = nc.dram_tensor("dbg_h3", [4 * 128, B_LOC * T], F32,
                                kind="ExternalOutput").ap()

    xr, o1r, o2r = x, o1, o2

    wT_names = [whh1T, wih2T, whh2T, wih3T, whh3T]

    with tile.TileContext(nc) as tc, ExitStack() as ctx:
        P = ctx.enter_context  # shorthand

        # ---------------- persistent SBUF ----------------
        wp = P(tc.tile_pool(name="wp", bufs=1))
        loopres_ctx = ExitStack()
        lrp = loopres_ctx.enter_context(tc.tile_pool(name="lrp", bufs=1))
        wT = [lrp.tile([128, G3], BF16, name=f"wT{i}_{k}", tag=f"wT{i}_{k}")
              for i in range(5) for k in range(4)]
        wT = [wT[i * 4:(i + 1) * 4] for i in range(5)]  # [matrix][k]
        gi1T = [lrp.tile([128, NTB], BF16, name=f"gi1T{c}", tag=f"gi1T{c}") for c in range(12)]
        h3T = [wp.tile([128, B_LOC * T], F32, name=f"h3T{k}", tag=f"h3T{k}") for k in range(4)]
        hstk = [wp.tile([128, H], F32, name=f"hstk{p}", tag=f"hstk{p}") for p in range(2)]
        bih1_sb = wp.tile([128, 12], F32, tag="bih1")
        brf_sb = wp.tile([65, 3 * H], F32, tag="brf")
        onesb_sb = wp.tile([65, 4], F32, tag="onesb")

        ones1_sb = wp.tile([1, 128], F32, tag="ones1")
        idb_sb = wp.tile([128, 128], BF16, tag="idb")
        idf_sb = wp.tile([128, 128], F32, tag="idf")

        nc.sync.dma_start(bih1_sb[:], bih1T)
        nc.sync.dma_start(brf_sb[:], brflat)
        nc.sync.dma_start(onesb_sb[:], onesb)
        nc.sync.dma_start(ones1_sb[:], ones1)
        nc.sync.dma_start(idb_sb[:], ident_bf)
        nc.sync.dma_start(idf_sb[:], ident_f32)
        for i in range(5):
            for k in range(4):
                nc.sync.dma_start(wT[i][k][:], wT_names[i][k * 128:(k + 1) * 128, :])

        # zero-init state
        for p in range(2):
            nc.vector.memset(hstk[p][:], 0.0)
        # ---------------- phases 0+1 (transient xT/wi1 pool) -----------------
        xp_ctx = ExitStack()
        xp = xp_ctx.enter_context(tc.tile_pool(name="xp", bufs=1))
        wi1 = [xp.tile([128, G3], BF16, name=f"wi1_{k}", tag=f"wi1_{k}") for k in range(5)]
        xT = [xp.tile([128, NTB], BF16, name=f"xT{k}", tag=f"xT{k}") for k in range(5)]
        for k in range(5):
            nc.sync.dma_start(wi1[k][:], wih1T[k * 128:(k + 1) * 128, :])
        nc.vector.memset(xT[4][:], 0.0)  # row 0 overwritten below

        # ---------------- phase 0: build xT (transposed x, bf16) -------------
        with tc.tile_pool(name="p0", bufs=3) as p0, \
             tc.tile_pool(name="p0ps", bufs=2, space="PSUM") as p0ps:
            for i in range(NTB // 128):
                xs = p0.tile([128, F_IN], F32, tag="xs")
                nc.sync.dma_start(xs[:], xr[i * 128:(i + 1) * 128, :])
                xb = p0.tile([128, 640], BF16, tag="xb")
                nc.scalar.copy(xb[:, 0:F_IN], xs[:])
                for c in range(5):
                    w_ = 128 if c < 4 else 1
                    tp = p0ps.tile([128, 128], BF16, tag="tp0")
                    nc.tensor.transpose(tp[0:w_, 0:128], xb[:, c * 128:c * 128 + w_],
                                        idb_sb[:])
                    nc.scalar.copy(xT[c][0:w_, i * 128:(i + 1) * 128], tp[0:w_, 0:128])

        # ---------------- phase 1: gi1T = w_ih1 @ x_t (+b_ih1), transposed ---
        with tc.tile_pool(name="p1ps", bufs=4, space="PSUM") as p1ps:
            for m in range(12):
                for n0 in range(0, NTB, 512):
                    w_ = min(512, NTB - n0)
                    ps = p1ps.tile([128, 512], F32, tag="ps1")
                    for k in range(5):
                        nc.tensor.matmul(ps[:, 0:w_], wi1[k][:, m * 128:(m + 1) * 128],
                                         xT[k][:, n0:n0 + w_],
                                         start=(k == 0), stop=(k == 4))
                    nc.scalar.activation(gi1T[m][:, n0:n0 + w_], ps[:, 0:w_],
                                         AF.Identity, bias=bih1_sb[:, m:m + 1])

        xp_ctx.close()

        # ---------------- phase 2: recurrence ----------------
        # Skew-2 wavefront: layer l processes t = s - 2l at step s, so the
        # cross-layer matmul inputs (gi2/gi3) come from TWO steps back and can
        # overlap the previous step's gate chain; only the self-recurrent gh
        # matmuls wait on the 1-back chain.
        loop_ctx = ExitStack()
        lp = loop_ctx.enter_context(tc.tile_pool(name="lp", bufs=2))
        psp = loop_ctx.enter_context(tc.tile_pool(name="psp", bufs=1, space="PSUM"))
        prz = [psp.tile([128, 1024], F32, name=f"prz{p}", tag=f"prz{p}")
               for p in range(2)]
        pgn = [psp.tile([128, 512], F32, name=f"pgn{p}", tag=f"pgn{p}")
               for p in range(2)]
        pin_ = psp.tile([128, 512], F32, name="pin", tag="pin")
        ptp = psp.tile([128, 512], F32, name="ptp", tag="ptp")
        for p in range(2):
            nc.vector.memset(prz[p][:], 0.0)
            nc.vector.memset(pgn[p][:], 0.0)
        nc.vector.memset(pin_[:], 0.0)
        nc.vector.memset(ptp[:], 0.0)
        hT3 = [[wp.tile([128, 68], BF16, name=f"hT3_{p}_{k}", tag=f"hT3_{p}_{k}")
                for k in range(4)] for p in range(3)]
        for p in range(3):
            for k in range(4):
                nc.vector.memset(hT3[p][k][:], 0.0)

        for s in range(T + 4):
            a1 = s <= T - 1
            a2 = 2 <= s <= T + 1
            a3 = 4 <= s <= T + 3
            b1m = (s - 1) % 3   # states written at s-1 (gh inputs)
            b2m = (s - 2) % 3   # states written at s-2 (gi inputs)
            pv = (s - 1) % 2
            cu = s % 2
            t1, t3 = s, s - 4

            ps_rz = prz[s % 2]
            ps_gn = pgn[s % 2]
            ps_in = pin_

            mms = []  # (region_key, dst, lhsT, rhs, tile_position, pri)
            # pri 0: no-dep / 2-back inputs first; pri 1: 1-back (gh) inputs
            if a1:
                for c in range(12):
                    if c < 8:
                        key, dst = ("rz", 0, (c // 4) * 512), ps_rz[0:4, c * 128:(c + 1) * 128]
                    else:
                        key, dst = ("in", 0, (c - 8) * 128), ps_in[0:4, (c - 8) * 128:(c - 7) * 128]
                    mms.append((key, dst, gi1T[c][:, 4 * t1:4 * t1 + 4],
                                idb_sb[:], (0, 0), 0))
                for k in range(4):
                    st = hT3[b1m][k][:, 0:4]
                    mms.append((("rz", 0, 0), ps_rz[0:4, 0:512], st,
                                wT[0][k][:, 0:512], (0, 0), 1))
                    mms.append((("rz", 0, 512), ps_rz[0:4, 512:1024], st,
                                wT[0][k][:, 512:1024], (0, 0), 1))
                    mms.append((("gn", 0, 0), ps_gn[0:4, :], st,
                                wT[0][k][:, 1024:1536], (0, 0), 1))
            if a2:
                for k in range(4):
                    s1_ = hT3[b2m][k][:, 0:4]    # h1(s-2)
                    s2_ = hT3[b1m][k][:, 32:36]  # h2(s-3)
                    mms.append((("rz", 32, 0), ps_rz[32:36, 0:512], s1_,
                                wT[1][k][:, 0:512], (0, 32), 0))
                    mms.append((("rz", 32, 512), ps_rz[32:36, 512:1024], s1_,
                                wT[1][k][:, 512:1024], (0, 32), 0))
                    mms.append((("in", 32, 0), ps_in[32:36, :], s1_,
                                wT[1][k][:, 1024:1536], (0, 32), 0))
                    mms.append((("rz", 32, 0), ps_rz[32:36, 0:512], s2_,
                                wT[2][k][:, 0:512], (0, 32), 1))
                    mms.append((("rz", 32, 512), ps_rz[32:36, 512:1024], s2_,
                                wT[2][k][:, 512:1024], (0, 32), 1))
                    mms.append((("gn", 32, 0), ps_gn[32:36, :], s2_,
                                wT[2][k][:, 1024:1536], (0, 32), 1))
            if a3:
                for k in range(4):
                    s2_ = hT3[b2m][k][:, 32:36]  # h2(s-4)
                    s3_ = hT3[b1m][k][:, 64:68]  # h3(s-5)
                    mms.append((("rz", 64, 0), ps_rz[64:68, 0:512], s2_,
                                wT[3][k][:, 0:512], (0, 64), 0))
                    mms.append((("rz", 64, 512), ps_rz[64:68, 512:1024], s2_,
                                wT[3][k][:, 512:1024], (0, 64), 0))
                    mms.append((("in", 64, 0), ps_in[64:68, :], s2_,
                                wT[3][k][:, 1024:1536], (0, 64), 0))
                    mms.append((("rz", 64, 0), ps_rz[64:68, 0:512], s3_,
                                wT[4][k][:, 0:512], (0, 64), 1))
                    mms.append((("rz", 64, 512), ps_rz[64:68, 512:1024], s3_,
                                wT[4][k][:, 512:1024], (0, 64), 1))
                    mms.append((("gn", 64, 0), ps_gn[64:68, :], s3_,
                                wT[4][k][:, 1024:1536], (0, 64), 1))
            bia = []
            if a1:
                bia.append((("gn", 0, 0), ps_gn[0:4, :], 0, (0, 0)))
            if a2:
                bia += [(("rz", 32, 0), ps_rz[32:36, 0:512], 1, (0, 32)),
                        (("rz", 32, 512), ps_rz[32:36, 512:1024], 2, (0, 32)),
                        (("gn", 32, 0), ps_gn[32:36, :], 3, (0, 32)),
                        (("in", 32, 0), ps_in[32:36, :], 4, (0, 32))]
            if a3:
                bia += [(("rz", 64, 0), ps_rz[64:68, 0:512], 5, (0, 64)),
                        (("rz", 64, 512), ps_rz[64:68, 512:1024], 6, (0, 64)),
                        (("gn", 64, 0), ps_gn[64:68, :], 7, (0, 64)),
                        (("in", 64, 0), ps_in[64:68, :], 8, (0, 64))]
            for key, dst, bi, tpos in bia:
                r_ = 32 * (bi // 3)
                c_ = bi % 3
                mms.append((key, dst, onesb_sb[r_:r_ + 1, 0:4],
                            brf_sb[r_:r_ + 1, c_ * H:(c_ + 1) * H],
                            (r_, tpos[1]), 0))
            # round-robin merge across col-group lanes, 2-back work first
            merged = []
            for pri in (0, 1):
                queues = {0: [], 1: [], 2: []}
                for m_ in mms:
                    if m_[5] == pri:
                        queues[m_[4][1] // 32].append(m_)
                qi = {g: 0 for g in queues}
                while True:
                    emitted = False
                    for g in (0, 1, 2):
                        if qi[g] < len(queues[g]):
                            merged.append(queues[g][qi[g]])
                            qi[g] += 1
                            emitted = True
                    if not emitted:
                        break
            started = set()
            last = {}
            for idx, (key, *_r) in enumerate(merged):
                last[key] = idx
            prev_mm = None
            for idx, (key, dst, lhsT, rhs, tpos, _pri) in enumerate(merged):
                st_flag = key not in started
                started.add(key)
                mi = nc.tensor.matmul(dst, lhsT, rhs, start=st_flag,
                                      stop=(last[key] == idx),
                                      tile_position=tpos, skip_group_check=True)
                prev_mm = mi.ins

            # gates (on [0:68]; inactive rows compute garbage that is unused)
            r_sb = lp.tile([128, 512], F32, tag="r_sb")
            nc.scalar.activation(r_sb[0:68, :], ps_rz[0:68, 0:512], AF.Sigmoid)
            z_sb = lp.tile([128, 512], F32, tag="z_sb")
            nc.scalar.activation(z_sb[0:68, :], ps_rz[0:68, 512:1024], AF.Sigmoid)
            omz = lp.tile([128, 512], F32, tag="omz")
            nc.vector.tensor_scalar(omz[0:68, :], z_sb[0:68, :], -1.0, 1.0,
                                    mybir.AluOpType.mult, mybir.AluOpType.add)
            zh = lp.tile([128, 512], F32, tag="zh")
            nc.vector.tensor_mul(zh[0:68, :], z_sb[0:68, :], hstk[pv][0:68, :])
            tmp = lp.tile([128, 512], F32, tag="tmp")
            nc.vector.tensor_mul(tmp[0:68, :], r_sb[0:68, :], ps_gn[0:68, :])
            npre = lp.tile([128, 512], F32, tag="npre")
            nc.vector.tensor_add(npre[0:68, :], tmp[0:68, :], ps_in[0:68, :])
            nsb = lp.tile([128, 512], F32, tag="nsb")
            nc.scalar.activation(nsb[0:68, :], npre[0:68, :], AF.Tanh)
            t1s = lp.tile([128, 512], F32, tag="t1s")
            nc.vector.tensor_mul(t1s[0:68, :], omz[0:68, :], nsb[0:68, :])
            nc.vector.tensor_add(hstk[cu][0:68, :], t1s[0:68, :], zh[0:68, :])

            # transpose h' chunks and store transposed states
            if s <= T + 2:
                tp = ptp
                for k in range(4):
                    nc.tensor.transpose(tp[0:128, k * 128:k * 128 + 68],
                                        hstk[cu][0:68, k * 128:(k + 1) * 128],
                                        idf_sb[0:68, 0:68])
                for k in range(4):
                    nc.scalar.copy(hT3[s % 3][k][:], tp[:, k * 128:k * 128 + 68])
                    if a3:
                        nc.vector.tensor_copy(h3T[k][:, 4 * t3:4 * t3 + 4],
                                              tp[:, k * 128 + 64:k * 128 + 68])
            elif a3:  # s == T+3
                tp = ptp
                for k in range(4):
                    nc.tensor.transpose(tp[0:128, k * 128 + 64:k * 128 + 68],
                                        hstk[cu][64:68, k * 128:(k + 1) * 128],
                                        idf_sb[64:68, 64:68])
                for k in range(4):
                    nc.vector.tensor_copy(h3T[k][:, 4 * t3:4 * t3 + 4],
                                          tp[:, k * 128 + 64:k * 128 + 68])

        loop_ctx.close()
        if not debug:
            loopres_ctx.close()

        if debug:
            with tc.tile_pool(name="dbgp", bufs=2) as dbgp:
                for c in range(12):
                    t_ = dbgp.tile([128, B_LOC * T], F32, tag="dbg_t")
                    nc.scalar.copy(t_[:], gi1T[c][:])
                    nc.sync.dma_start(dbg_gi[c * 128:(c + 1) * 128, :], t_[:])
                for k in range(4):
                    t_ = dbgp.tile([128, B_LOC * T], F32, tag="dbg_t2")
                    nc.scalar.copy(t_[:], h3T[k][:])
                    nc.sync.dma_start(dbg_h3[k * 128:(k + 1) * 128, :], t_[:])
            loopres_ctx.close()

        # ---------------- phase 3: heads + masks ----------------
        with tc.tile_pool(name="p3", bufs=3) as p3, \
             tc.tile_pool(name="p3ps", bufs=4, space="PSUM") as p3ps:
            wl = [[p3.tile([128, F_IN], F32, name=f"wl{h}_{k}", tag=f"wl{h}_{k}",
                           bufs=1) for k in range(4)] for h in range(2)]
            bl_sb = [p3.tile([1, F_IN], F32, name=f"bl_{h}", tag=f"bl_{h}", bufs=1)
                     for h in range(2)]
            nc.sync.dma_start(bl_sb[0][:], bl1)
            nc.sync.dma_start(bl_sb[1][:], bl2)
            for h, w_ap in ((0, wl1T), (1, wl2T)):
                for k in range(4):
                    nc.sync.dma_start(wl[h][k][:], w_ap[k * 128:(k + 1) * 128, :])
            for i in range(NTB // 128):
                xs = p3.tile([128, F_IN], F32, tag="x3")
                nc.sync.dma_start(xs[:], xr[i * 128:(i + 1) * 128, :])
                ssb = []
                for h in range(2):
                    ps = p3ps.tile([128, 512], F32, tag="ps3")
                    psb = p3ps.tile([128, 8], F32, tag="ps3b")
                    for k in range(4):
                        st = h3T[k][:, 128 * i: 128 * (i + 1)]
                        nc.tensor.matmul(ps[:], st, wl[h][k][:, 0:512],
                                         start=(k == 0), stop=False)
                        nc.tensor.matmul(psb[:, 0:1], st, wl[h][k][:, 512:513],
                                         start=(k == 0), stop=False)
                    nc.tensor.matmul(ps[:], ones1_sb[:], bl_sb[h][:, 0:512],
                                     start=False, stop=True)
                    nc.tensor.matmul(psb[:, 0:1], ones1_sb[:],
                                     bl_sb[h][:, 512:513],
                                     start=False, stop=True)
                    sb = p3.tile([128, F_IN], F32, tag=f"s{h}")
                    nc.scalar.activation(sb[:, 0:512], ps[:], AF.Relu)
                    nc.scalar.activation(sb[:, 512:513], psb[:, 0:1], AF.Relu)
                    ssb.append(sb)
                den = p3.tile([128, F_IN], F32, tag="den")
                nc.vector.tensor_add(den[:], ssb[0][:], ssb[1][:])
                nc.vector.tensor_scalar_add(den[:], den[:], 1e-16)
                rec = p3.tile([128, F_IN], F32, tag="rec")
                nc.vector.reciprocal(rec[:], den[:])
                rx = p3.tile([128, F_IN], F32, tag="rx")
                nc.vector.tensor_mul(rx[:], rec[:], xs[:])
                for h, outr in ((0, o1r), (1, o2r)):
                    ob = p3.tile([128, F_IN], F32, tag=f"ob{h}")
                    nc.vector.tensor_mul(ob[:], ssb[h][:], rx[:])
                    nc.sync.dma_start(outr[i * 128:(i + 1) * 128, :], ob[:])

    return nc


_CACHE = {}


def _get_nc(T, debug=False):
    key = (T, debug)
    if key not in _CACHE:
        _CACHE[key] = _build(T, debug)
    return _CACHE[key]


def _get_nc_split(T):
    """nc with the walrus single-wait fixup applied (hardware path)."""
    nc = _get_nc(T)
    if not getattr(nc, "_waits_split", False):
        _split_multi_waits(nc)
        nc._waits_split = True
    return nc


def _prep_shared(weights, T):
    w = {k: np.asarray(v, dtype=np.float32) for k, v in weights.items()}
    bf = ml_dtypes.bfloat16
    out = {}
    wih1T = np.zeros((640, G3), np.float32)
    wih1T[:F_IN, :] = w["w_ih1"].T
    out["wih1T"] = wih1T.astype(bf)
    for nm, key in (("whh1T", "w_hh1"), ("wih2T", "w_ih2"), ("whh2T", "w_hh2"),
                    ("wih3T", "w_ih3"), ("whh3T", "w_hh3")):
        out[nm] = np.ascontiguousarray(w[key].T).astype(bf)
    out["wl1T"] = np.ascontiguousarray(w["w_l1"].T).astype(np.float32)
    out["wl2T"] = np.ascontiguousarray(w["w_l2"].T).astype(np.float32)
    comb = w["b_ih1"].copy()
    comb[0:2 * H] += w["b_hh1"][0:2 * H]   # L1 rz bias rides the gi1 injection
    out["bih1T"] = np.ascontiguousarray(comb.reshape(12, 128).T).astype(np.float32)
    b_hh1, b_hh2, b_hh3 = w["b_hh1"], w["b_hh2"], w["b_hh3"]
    b_ih2, b_ih3 = w["b_ih2"], w["b_ih3"]
    rows = [
        b_hh1[2*H:],                                   # (L1, gn)
        b_ih2[0:H] + b_hh2[0:H], b_ih2[H:2*H] + b_hh2[H:2*H],
        b_hh2[2*H:], b_ih2[2*H:],                      # L2: r, z, gn, in
        b_ih3[0:H] + b_hh3[0:H], b_ih3[H:2*H] + b_hh3[H:2*H],
        b_hh3[2*H:], b_ih3[2*H:],                      # L3: r, z, gn, in
    ]
    brf = np.zeros((65, 3 * H), np.float32)
    for bi, row in enumerate(rows):
        brf[32 * (bi // 3), (bi % 3) * H:(bi % 3 + 1) * H] = row
    out["brflat"] = brf
    onesb = np.zeros((65, 4), np.float32)
    onesb[[0, 32, 64], :] = 1.0
    out["onesb"] = onesb
    out["bl1"] = w["b_l1"].reshape(1, -1).astype(np.float32)
    out["bl2"] = w["b_l2"].reshape(1, -1).astype(np.float32)
    out["ones1"] = np.ones((1, 128), np.float32)
    out["ident_bf"] = np.eye(128, dtype=np.float32).astype(bf)
    out["ident_f32"] = np.eye(128, dtype=np.float32)
    return out


def kernel(**inputs):
    x_full = np.asarray(inputs["x"], dtype=np.float32)
    Bx, T, F = x_full.shape
    assert (Bx, F) == (B, F_IN)
    weights = {k: v for k, v in inputs.items() if k != "x"}
    nc = _get_nc_split(T)
    shared = _prep_shared(weights, T)
    in_maps = []
    for c in range(N_CORES):
        m = dict(shared)
        xc = x_full[c * B_LOC:(c + 1) * B_LOC]            # [4, T, F]
        m["x"] = np.ascontiguousarray(xc.transpose(1, 0, 2).reshape(B_LOC * T, F_IN))
        in_maps.append(m)
    res = run_bass_kernel_spmd(nc, in_maps, list(range(N_CORES)))

    def unpack(name):
        parts = [r[name].reshape(T, B_LOC, F_IN).transpose(1, 0, 2)
                 for r in res.results]
        return np.concatenate(parts, axis=0)

    return unpack("o1"), unpack("o2")
